# revision 30
# baseline (speedup 1.0000x reference)
"""Trainium2 Bass kernel for nn_Complex_Only_46308337385506 (gnn_message_passing).

Math (derived + numerically validated against the jax reference):
  The per-edge orthonormal basis R (rows nU, nV, nJ) enters the output only
  through two per-edge scalars:
      gam = Jz*t,  t = 1/|J|
      a1  = sqrt(Jx^2+Jy^2)*t * sign(Jz+eps)
  With wt = gam*Xz - a1*Xx:
      Y0 = Wa@Xx + (Wa-Wc)@(a1*wt) + Wb@(gam*Xy)
      Y1 = Wa@Xy - Wb@(a1*Xz + gam*Xx)
      Y2 = Wa@Xz + (Wc-Wa)@(gam*wt) + Wb@(a1*Xy)
  followed by the VN leaky-relu stage:
      d = Wd@Y (over channel dim), dot = <Y,d>_3, dn2 = <d,d>_3
      out = Y - 0.8*min(dot,0)/(dn2+eps) * d

Sharding: data-parallel over batch B=8 -> one batch per NeuronCore.

Perf design (cost-model ~186us baseline -> target ~130us):
  - bf16 everywhere the 2e-2 tolerance allows (validated 6.2e-3 norm-rel in
    numpy): inputs are cast to bf16 host-side AND row-permuted so each
    partition's 8 point-rows are contiguous in HBM (3072B descriptors =
    full-speed DMA); output is written bf16 (1024B descriptors) and upcast
    host-side. DMA/super drops 6552ns -> 3276ns.
  - bf16 products/transposes: PE transposes cost 1 cyc/col, PSUM->SBUF
    copies hit the DVE 2x_1p mode (401ns vs 667 for f32).
  - prod slots are pre-paired (Xx,p2 | Xz,p3 | xyb,c8 | p4,p5) so each
    K=128 stationary combines two weight blocks; Y lands directly in
    xsb-layout [128=(g,f), 3, 512] PSUM via M=64 matmuls at partition
    offset 64g (5 matmuls/group), letting ONE wide ACT copy produce xsb.
  - VN scale uses TT-divide on Pool (no reciprocal+STT chain).
  - Op placement balances DVE (bf16 2x/4x ops, customs), ACT (sqrt/sign,
    wide f32 PSUM copies), Pool (flat-rate mixed-dtype muls, divide).
"""

import numpy as np
from contextlib import ExitStack

import concourse.bass as bass
import concourse.bacc as bacc
import concourse.tile as tile
from concourse import mybir
from concourse import bass_utils

F32 = mybir.dt.float32
BF16 = mybir.dt.bfloat16
AF = mybir.ActivationFunctionType
ALU = mybir.AluOpType

EPS = 1e-6
NEG = 0.2

B, C, E = 8, 16384, 64
SUPER = 1024           # points per super-iteration
NSUP = C // SUPER      # 16
GROUP = 512            # matmul free dim (points)
NCHUNK = 8             # 128-pt chunks per super


_CUSTOM_OPS = {}


def _register_custom_dve_ops():
    """Fused DVE ops (module-level, idempotent):
      SQSUM_ANT:  out = Src0^2 + Src1^2
      ADDSQ_ANT:  out = Src0 + Src1^2
      ADDSQS_ANT: out = (Src0 + Src1^2) * s0
    """
    if _CUSTOM_OPS:
        return _CUSTOM_OPS
    import numpy as _np
    from concourse import dve_ops
    from concourse.dve_spec import Spec, Src0, Src1, lower, sq, _has_src1

    def make(name, body, ref):
        spec = Spec(body=body, reference=ref)
        opcode = dve_ops._CUSTOM_DVE_ROW_BASE + len(dve_ops.OPS)
        shas = {}
        from concourse.dve_uop import DveOpSpec
        from concourse.dve_table_gen import dve_ver_for
        for ver in ("v3", "v4"):
            try:
                s = DveOpSpec(name=name, opcode=opcode,
                              uops=lower(spec, ver=ver),
                              rd1_en=_has_src1(spec))
                shas[ver] = s.sha(ver)
            except Exception:
                pass
        op = dve_ops.DveOp(name, spec, subdim=False, uops_sha=shas)
        dve_ops.OPS.append(op)
        dve_ops.CUSTOM_DVE_SPECS[name] = spec
        dve_ops._SUB_OPCODE_FOR_NAME[name] = opcode
        assert opcode < 0x20
        return op

    def fl(a):
        # operand APs may lower with different (collapsed vs not) free-dim
        # shapes; flatten to [p, -1] (same row-major order) before combining
        return _np.asarray(a).reshape(_np.asarray(a).shape[0], -1)

    _CUSTOM_OPS["SQSUM"] = make(
        "SQSUM_ANT", sq(Src0) + sq(Src1),
        lambda in0, in1, s0, s1, imm2:
            (fl(in0).astype(_np.float32) ** 2 + fl(in1).astype(_np.float32) ** 2))
    _CUSTOM_OPS["ADDSQ"] = make(
        "ADDSQ_ANT", Src0 + sq(Src1),
        lambda in0, in1, s0, s1, imm2:
            fl(in0).astype(_np.float32) + fl(in1).astype(_np.float32) ** 2)
    from concourse.dve_spec import C0, C1, Zero, minn, select
    _CUSTOM_OPS["ADDSQSE"] = make(
        "ADDSQSE_ANT", (Src0 + sq(Src1)) * C0 + C1,
        lambda in0, in1, s0, s1, imm2:
            (fl(in0).astype(_np.float32) + fl(in1).astype(_np.float32) ** 2)
            * s0 + s1)
    _CUSTOM_OPS["CPSGN"] = make(
        "CPSGN_ANT", select((Src1 + C0) >= Zero, Src0, Zero - Src0),
        lambda in0, in1, s0, s1, imm2:
            _np.where(fl(in1).astype(_np.float32) + s0 >= 0,
                      fl(in0).astype(_np.float32),
                      -fl(in0).astype(_np.float32)))
    _CUSTOM_OPS["MINADD"] = make(
        "MINADD_ANT", minn(Src0 + Src1, Zero),
        lambda in0, in1, s0, s1, imm2:
            _np.minimum(fl(in0).astype(_np.float32)
                        + fl(in1).astype(_np.float32), 0.0))
    return _CUSTOM_OPS


def _pin_act_table_set(arch: str):
    """Steer the ACT table-set chooser: all funcs this kernel uses must
    first-match sqrt_and_others, so exactly one table load is emitted."""
    from concourse import hw_specs
    tables = hw_specs.get_activation_tables(arch)  # cached dict, mutate in place
    mine = {AF.Sqrt, AF.Sign, AF.Copy, AF.Identity, AF.Square}
    for name, funcs in tables.items():
        if name != "sqrt_and_others":
            funcs -= mine


def _build_nc():
    global OPS
    OPS = _register_custom_dve_ops()
    nc = bacc.Bacc("TRN2", debug=False)
    _pin_act_table_set(nc.m.arch)

    XS = nc.dram_tensor("XS", [NSUP, 128, NCHUNK, 3, E], BF16,
                        kind="ExternalInput").ap()
    JS = nc.dram_tensor("JS", [NSUP, 128, NCHUNK, 3, E], BF16,
                        kind="ExternalInput").ap()
    WMM = nc.dram_tensor("WMM", [5, 128, 128], F32, kind="ExternalInput").ap()
    OUT = nc.dram_tensor("OUT", [64, 3, C], BF16, kind="ExternalOutput").ap()

    with tile.TileContext(nc) as tc, ExitStack() as ctx:
        const = ctx.enter_context(tc.tile_pool(name="const", bufs=1))
        io = ctx.enter_context(tc.tile_pool(name="io", bufs=3))
        sa = ctx.enter_context(tc.tile_pool(name="sa", bufs=3))
        prodp = ctx.enter_context(tc.tile_pool(name="prodp", bufs=3))
        rhsp = ctx.enter_context(tc.tile_pool(name="rhsp", bufs=3))
        s3p = ctx.enter_context(tc.tile_pool(name="s3p", bufs=3))
        outp = ctx.enter_context(tc.tile_pool(name="outp", bufs=3))
        psT = ctx.enter_context(tc.tile_pool(name="psT", bufs=1, space="PSUM"))
        psY = ctx.enter_context(tc.tile_pool(name="psY", bufs=1, space="PSUM"))
        psD = ctx.enter_context(tc.tile_pool(name="psD", bufs=1, space="PSUM"))

        eps_c = const.tile([128, 1], F32, tag="eps_c")
        nc.gpsimd.memset(eps_c[:], EPS)

        # weights: f32 load once, cast to bf16 (one ACT pass)
        wsb = const.tile([128, 5, 128], F32)
        nc.sync.dma_start(wsb[:], WMM.rearrange("n p m -> p n m"))
        wbf = const.tile([128, 5, 128], BF16)
        nc.scalar.activation(wbf[:], wsb[:], AF.Copy)
        W0 = wbf[:, 0, :]      # [WaT;W2nT | WaT;W2T]  (cols 0:64 -> Y0, 64:128 -> Y2)
        W1 = wbf[:, 1, :]      # blkdiag(WbT, WbT)     (cols 0:64 -> Y0, 64:128 -> Y2)
        W2s = wbf[:, 2, 0:64]  # [WaT; -WbT]           (-> Y1)
        W3 = wbf[:, 3, :]      # blkdiag(WdT, WdT)
        IDb = wbf[:, 4, :]     # identity (bf16 transposes)

        for u in range(NSUP):
            jst = io.tile([128, NCHUNK * 192], BF16, tag="js")
            nc.sync.dma_start(
                jst[:].rearrange("p (s c e) -> p s c e", s=NCHUNK, c=3, e=E),
                JS[u])
            # host layout is [s, c, e]: every field slice is packed (innermost e)
            jv = jst[:].rearrange("p (s c e) -> p s c e", s=NCHUNK, c=3, e=E)
            # X fields land straight in their prod slots (slot = c: 0=Xx 2=Xz
            # 4=xyb), skipping the SBUF staging copy
            prod = prodp.tile([128, NCHUNK, 8, E], BF16, tag="prod")
            for ci in range(3):
                nc.sync.dma_start(prod[:, :, 2 * ci, :], XS[u, :, :, ci, :])

            def v3(t):  # [128, 512] tile -> [128, 8, 64] view
                return t[:].rearrange("p (s e) -> p s e", s=NCHUNK, e=E)

            # ---- stage A: per-edge scalars gam, a1 ---------------------
            qn2 = sa.tile([128, 2, SUPER // 2], BF16, tag="qn2")
            q = qn2[:, 0, :]
            n2 = qn2[:, 1, :]
            nc.vector._custom_dve(OPS["SQSUM"], out=q,
                                  in0=jv[:, :, 0, :], in1=jv[:, :, 1, :]
                                  ).ins.perf_max = 2
            nc.vector._custom_dve(OPS["ADDSQ"], out=n2,
                                  in0=q, in1=jv[:, :, 2, :]
                                  ).ins.perf_max = 2
            sqs = sa.tile([128, 2, SUPER // 2], BF16, tag="sqs")
            nc.scalar.activation(sqs[:], qn2[:], AF.Sqrt)
            sq_ = sqs[:, 0, :]
            s_ = sqs[:, 1, :]
            gam = sa.tile([128, SUPER // 2], BF16, tag="gam")
            nc.gpsimd.tensor_tensor(v3(gam), jv[:, :, 2, :],
                                    s_.rearrange("p (s e) -> p s e", s=NCHUNK, e=E),
                                    ALU.divide)
            a1u = sa.tile([128, SUPER // 2], BF16, tag="a1u")
            nc.gpsimd.tensor_tensor(a1u[:], sq_, s_, ALU.divide)
            a1 = sa.tile([128, SUPER // 2], BF16, tag="a1")
            nc.vector._custom_dve(OPS["CPSGN"], out=a1[:], in0=a1u[:],
                                  in1=jv[:, :, 2, :], s0=EPS).ins.perf_max = 2

            # ---- stage B: products into paired prod slots --------------
            # slots: 0=Xx 1=p2 2=xyb 3=c8 4=Xz 5=p3 6=p4 7=p5
            m1 = sa.tile([128, SUPER // 2], BF16, tag="m1")
            m2 = sa.tile([128, SUPER // 2], BF16, tag="m2")
            m3 = sa.tile([128, SUPER // 2], BF16, tag="m3")
            m4 = sa.tile([128, SUPER // 2], BF16, tag="m4")
            nc.gpsimd.tensor_tensor(v3(m1), v3(gam), prod[:, :, 4, :], ALU.mult)
            nc.gpsimd.tensor_tensor(v3(m2), v3(a1), prod[:, :, 0, :], ALU.mult)
            nc.gpsimd.tensor_tensor(v3(m3), v3(a1), prod[:, :, 4, :], ALU.mult)
            nc.gpsimd.tensor_tensor(v3(m4), v3(gam), prod[:, :, 0, :], ALU.mult)
            wt = sa.tile([128, SUPER // 2], BF16, tag="wt")
            nc.vector.tensor_tensor(wt[:], m1[:], m2[:], ALU.subtract)
            nc.vector.tensor_tensor(prod[:, :, 3, :], v3(m3), v3(m4), ALU.add)
            nc.vector.tensor_tensor(prod[:, :, 1, :], v3(a1), v3(wt), ALU.mult)
            nc.vector.tensor_tensor(prod[:, :, 5, :], v3(gam), v3(wt), ALU.mult)
            nc.vector.tensor_tensor(prod[:, :, 6, :], v3(gam), prod[:, :, 2, :],
                                    ALU.mult)
            nc.gpsimd.tensor_tensor(prod[:, :, 7, :], v3(a1), prod[:, :, 2, :],
                                    ALU.mult)

            # ---- per group: transpose, rh copies, Y matmuls ------------
            pY = psY.tile([128, 3, GROUP], F32, tag="pY")
            for g in range(2):
                tpA = psT.tile([128, 2, GROUP], BF16, tag="tpA")
                tpB = psT.tile([128, 2, GROUP], BF16, tag="tpB")
                for k in range(4):
                    s = 4 * g + k
                    sl = slice(128 * k, 128 * (k + 1))
                    nc.tensor.transpose(tpA[:, 0, sl], prod[:, s, 0:2, :], IDb)
                    nc.tensor.transpose(tpA[:, 1, sl], prod[:, s, 4:6, :], IDb)
                    nc.tensor.transpose(tpB[:, 0, sl], prod[:, s, 2:4, :], IDb)
                    nc.tensor.transpose(tpB[:, 1, sl], prod[:, s, 6:8, :], IDb)
                rhA = rhsp.tile([128, 2, GROUP], BF16, tag="rhA")
                rhB = rhsp.tile([128, 2, GROUP], BF16, tag="rhB")
                if g == 0:
                    nc.vector.tensor_copy(rhA[:], tpA[:])
                    nc.vector.tensor_copy(rhB[:], tpB[:])
                else:
                    nc.scalar.activation(rhA[:], tpA[:], AF.Copy)
                    nc.scalar.activation(rhB[:], tpB[:], AF.Copy)

                ro = slice(64 * g, 64 * (g + 1))
                rh1 = rhA[:, 0, :]   # [Xx; p2]
                rh2 = rhA[:, 1, :]   # [Xz; p3]
                rh3 = rhB[:, 0, :]   # [xyb; c8]
                rh4 = rhB[:, 1, :]   # [p4; p5]
                nc.tensor.matmul(pY[ro, 0, :], W0[:, 0:64], rh1,
                                 start=True, stop=False)
                nc.tensor.matmul(pY[ro, 0, :], W1[:, 0:64], rh4,
                                 start=False, stop=True)
                nc.tensor.matmul(pY[ro, 2, :], W0[:, 64:128], rh2,
                                 start=True, stop=False)
                nc.tensor.matmul(pY[ro, 2, :], W1[:, 64:128], rh4,
                                 start=False, stop=True)
                nc.tensor.matmul(pY[ro, 1, :], W2s, rh3, start=True, stop=True)

            # ---- Wd stage ----------------------------------------------
            xsb = s3p.tile([128, 3, GROUP], BF16, tag="xsb")
            nc.scalar.activation(xsb[:], pY[:], AF.Copy)
            pd = psD.tile([128, 3, GROUP], F32, tag="pd")
            for i in range(3):
                nc.tensor.matmul(pd[:, i, :], W3, xsb[:, i, :], start=True,
                                 stop=True)
            dsb = s3p.tile([128, 3, GROUP], BF16, tag="dsb")
            nc.scalar.activation(dsb[:], pd[:], AF.Copy)

            # ---- VN leaky relu -----------------------------------------
            P = s3p.tile([128, 3, GROUP], BF16, tag="P")
            nc.vector.tensor_tensor(P[:], xsb[:], dsb[:], ALU.mult)
            dot = s3p.tile([128, GROUP], BF16, tag="dot")
            nc.vector.tensor_tensor(dot[:], P[:, 0, :], P[:, 1, :], ALU.add)
            dotm = s3p.tile([128, GROUP], BF16, tag="dotm")
            nc.vector._custom_dve(OPS["MINADD"], out=dotm[:],
                                  in0=dot[:], in1=P[:, 2, :]).ins.perf_max = 2

            dn2 = s3p.tile([128, GROUP], BF16, tag="dn2")
            nc.vector._custom_dve(OPS["SQSUM"], out=dn2[:],
                                  in0=dsb[:, 0, :], in1=dsb[:, 1, :]
                                  ).ins.perf_max = 2
            dn2f = s3p.tile([128, GROUP], BF16, tag="dn2f")
            nc.vector._custom_dve(OPS["ADDSQSE"], out=dn2f[:],
                                  in0=dn2[:], in1=dsb[:, 2, :],
                                  s0=1.0 / (1.0 - NEG),
                                  s1=EPS / (1.0 - NEG)).ins.perf_max = 2
            s2 = s3p.tile([128, GROUP], BF16, tag="s2")
            nc.gpsimd.tensor_tensor(s2[:], dotm[:], dn2f[:], ALU.divide)

            mi3 = s3p.tile([128, 3, GROUP], BF16, tag="mi3")
            for i in range(3):
                nc.gpsimd.tensor_tensor(mi3[:, i, :], s2[:], dsb[:, i, :],
                                        ALU.mult)
            ot = outp.tile([128, 3, GROUP], BF16, tag="ot")
            nc.gpsimd.tensor_tensor(ot[:], xsb[:], mi3[:], ALU.subtract)

            c0 = u * SUPER
            nc.sync.dma_start(OUT[:, :, c0:c0 + GROUP], ot[0:64])
            nc.sync.dma_start(OUT[:, :, c0 + GROUP:c0 + SUPER], ot[64:128])

    nc.compile()
    return nc


_NC = None


def _get_nc():
    global _NC
    if _NC is None:
        _NC = _build_nc()
    return _NC


def _weight_stack(Wa, Wb, Wc, Wd):
    Z = np.zeros((64, 64), np.float32)
    WaT = Wa.T.astype(np.float32)
    WbT = Wb.T.astype(np.float32)
    W2nT = (Wa - Wc).T.astype(np.float32)
    W2T = (Wc - Wa).T.astype(np.float32)
    WdT = Wd.T.astype(np.float32)

    def vs(a, b):
        return np.vstack([a, b]).astype(np.float32)   # [128, 64]

    s0 = np.hstack([vs(WaT, W2nT), vs(WaT, W2T)])     # [128,128]
    s1 = np.block([[WbT, Z], [Z, WbT]]).astype(np.float32)
    s2 = np.hstack([vs(WaT, -WbT), np.zeros((128, 64), np.float32)])
    s3 = np.block([[WdT, Z], [Z, WdT]]).astype(np.float32)
    s4 = np.eye(128, dtype=np.float32)
    return np.ascontiguousarray(np.stack([s0, s1, s2, s3, s4]), np.float32)


def _prep_input(A):
    """[C, E, 3] f32 -> [NSUP, 128, NCHUNK*192] bf16, [s, c, e]-ordered per
    point so every field slice is packed, with each partition's 8 point-rows
    contiguous in HBM (3072B DMA descriptors)."""
    import ml_dtypes
    Ap = A.reshape(NSUP, NCHUNK, 128, E, 3).transpose(0, 2, 1, 4, 3)
    Ap = np.ascontiguousarray(Ap)
    return Ap.astype(ml_dtypes.bfloat16)


def run_full(X, J, Wa, Wb, Wc, Wd, trace=False, trace_kwargs=None):
    nc = _get_nc()
    wmm = _weight_stack(Wa, Wb, Wc, Wd)
    in_maps = []
    for b in range(B):
        in_maps.append({
            "XS": _prep_input(np.asarray(X[b], np.float32)),
            "JS": _prep_input(np.asarray(J[b], np.float32)),
            "WMM": wmm,
        })
    res = bass_utils.run_bass_kernel_spmd(
        nc, in_maps, core_ids=list(range(B)), trace=trace,
        **(trace_kwargs or {}))
    out = np.stack([np.asarray(res.results[b]["OUT"]).astype(np.float32)
                    for b in range(B)])
    return out, res


def kernel(X, J, Wa, Wb, Wc, Wd):
    out, _ = run_full(X, J, Wa, Wb, Wc, Wd)
    return out


# revision 33
# speedup vs baseline: 1.0139x; 1.0139x over previous
"""Trainium2 Bass kernel for nn_Complex_Only_46308337385506 (gnn_message_passing).

Math (derived + numerically validated against the jax reference):
  The per-edge orthonormal basis R (rows nU, nV, nJ) enters the output only
  through two per-edge scalars:
      gam = Jz*t,  t = 1/|J|
      a1  = sqrt(Jx^2+Jy^2)*t * sign(Jz+eps)
  With wt = gam*Xz - a1*Xx:
      Y0 = Wa@Xx + (Wa-Wc)@(a1*wt) + Wb@(gam*Xy)
      Y1 = Wa@Xy - Wb@(a1*Xz + gam*Xx)
      Y2 = Wa@Xz + (Wc-Wa)@(gam*wt) + Wb@(a1*Xy)
  followed by the VN leaky-relu stage:
      d = Wd@Y (over channel dim), dot = <Y,d>_3, dn2 = <d,d>_3
      out = Y - 0.8*min(dot,0)/(dn2+eps) * d

Sharding: data-parallel over batch B=8 -> one batch per NeuronCore.

Perf design (cost-model ~186us baseline -> target ~130us):
  - bf16 everywhere the 2e-2 tolerance allows (validated 6.2e-3 norm-rel in
    numpy): inputs are cast to bf16 host-side AND row-permuted so each
    partition's 8 point-rows are contiguous in HBM (3072B descriptors =
    full-speed DMA); output is written bf16 (1024B descriptors) and upcast
    host-side. DMA/super drops 6552ns -> 3276ns.
  - bf16 products/transposes: PE transposes cost 1 cyc/col, PSUM->SBUF
    copies hit the DVE 2x_1p mode (401ns vs 667 for f32).
  - prod slots are pre-paired (Xx,p2 | Xz,p3 | xyb,c8 | p4,p5) so each
    K=128 stationary combines two weight blocks; Y lands directly in
    xsb-layout [128=(g,f), 3, 512] PSUM via M=64 matmuls at partition
    offset 64g (5 matmuls/group), letting ONE wide ACT copy produce xsb.
  - VN scale uses TT-divide on Pool (no reciprocal+STT chain).
  - Op placement balances DVE (bf16 2x/4x ops, customs), ACT (sqrt/sign,
    wide f32 PSUM copies), Pool (flat-rate mixed-dtype muls, divide).
"""

import numpy as np
from contextlib import ExitStack

import concourse.bass as bass
import concourse.bacc as bacc
import concourse.tile as tile
from concourse import mybir
from concourse import bass_utils

F32 = mybir.dt.float32
BF16 = mybir.dt.bfloat16
AF = mybir.ActivationFunctionType
ALU = mybir.AluOpType

EPS = 1e-6
NEG = 0.2

B, C, E = 8, 16384, 64
SUPER = 1024           # points per super-iteration
NSUP = C // SUPER      # 16
GROUP = 512            # matmul free dim (points)
NCHUNK = 8             # 128-pt chunks per super


_CUSTOM_OPS = {}


def _register_custom_dve_ops():
    """Fused DVE ops (module-level, idempotent):
      SQSUM_ANT:  out = Src0^2 + Src1^2
      ADDSQ_ANT:  out = Src0 + Src1^2
      ADDSQS_ANT: out = (Src0 + Src1^2) * s0
    """
    if _CUSTOM_OPS:
        return _CUSTOM_OPS
    import numpy as _np
    from concourse import dve_ops
    from concourse.dve_spec import Spec, Src0, Src1, lower, sq, _has_src1

    def make(name, body, ref):
        spec = Spec(body=body, reference=ref)
        opcode = dve_ops._CUSTOM_DVE_ROW_BASE + len(dve_ops.OPS)
        shas = {}
        from concourse.dve_uop import DveOpSpec
        from concourse.dve_table_gen import dve_ver_for
        for ver in ("v3", "v4"):
            try:
                s = DveOpSpec(name=name, opcode=opcode,
                              uops=lower(spec, ver=ver),
                              rd1_en=_has_src1(spec))
                shas[ver] = s.sha(ver)
            except Exception:
                pass
        op = dve_ops.DveOp(name, spec, subdim=False, uops_sha=shas)
        dve_ops.OPS.append(op)
        dve_ops.CUSTOM_DVE_SPECS[name] = spec
        dve_ops._SUB_OPCODE_FOR_NAME[name] = opcode
        assert opcode < 0x20
        return op

    def fl(a):
        # operand APs may lower with different (collapsed vs not) free-dim
        # shapes; flatten to [p, -1] (same row-major order) before combining
        return _np.asarray(a).reshape(_np.asarray(a).shape[0], -1)

    _CUSTOM_OPS["SQSUM"] = make(
        "SQSUM_ANT", sq(Src0) + sq(Src1),
        lambda in0, in1, s0, s1, imm2:
            (fl(in0).astype(_np.float32) ** 2 + fl(in1).astype(_np.float32) ** 2))
    _CUSTOM_OPS["ADDSQ"] = make(
        "ADDSQ_ANT", Src0 + sq(Src1),
        lambda in0, in1, s0, s1, imm2:
            fl(in0).astype(_np.float32) + fl(in1).astype(_np.float32) ** 2)
    from concourse.dve_spec import C0, C1, Zero, minn, select
    _CUSTOM_OPS["ADDSQSE"] = make(
        "ADDSQSE_ANT", (Src0 + sq(Src1)) * C0 + C1,
        lambda in0, in1, s0, s1, imm2:
            (fl(in0).astype(_np.float32) + fl(in1).astype(_np.float32) ** 2)
            * s0 + s1)
    _CUSTOM_OPS["CPSGN"] = make(
        "CPSGN_ANT", select((Src1 + C0) >= Zero, Src0, Zero - Src0),
        lambda in0, in1, s0, s1, imm2:
            _np.where(fl(in1).astype(_np.float32) + s0 >= 0,
                      fl(in0).astype(_np.float32),
                      -fl(in0).astype(_np.float32)))
    _CUSTOM_OPS["MINADD"] = make(
        "MINADD_ANT", minn(Src0 + Src1, Zero),
        lambda in0, in1, s0, s1, imm2:
            _np.minimum(fl(in0).astype(_np.float32)
                        + fl(in1).astype(_np.float32), 0.0))
    return _CUSTOM_OPS


def _pin_act_table_set(arch: str):
    """Steer the ACT table-set chooser: all funcs this kernel uses must
    first-match sqrt_and_others, so exactly one table load is emitted."""
    from concourse import hw_specs
    tables = hw_specs.get_activation_tables(arch)  # cached dict, mutate in place
    mine = {AF.Sqrt, AF.Sign, AF.Copy, AF.Identity, AF.Square}
    for name, funcs in tables.items():
        if name != "sqrt_and_others":
            funcs -= mine


def _build_nc():
    global OPS
    OPS = _register_custom_dve_ops()
    nc = bacc.Bacc("TRN2", debug=False)
    _pin_act_table_set(nc.m.arch)

    XS = nc.dram_tensor("XS", [NSUP, 128, NCHUNK * 192], BF16,
                        kind="ExternalInput").ap()
    JS = nc.dram_tensor("JS", [NSUP, 128, NCHUNK * 192], BF16,
                        kind="ExternalInput").ap()
    WMM = nc.dram_tensor("WMM", [5, 128, 128], F32, kind="ExternalInput").ap()
    OUT = nc.dram_tensor("OUT", [64, 3, C], BF16, kind="ExternalOutput").ap()

    with tile.TileContext(nc) as tc, ExitStack() as ctx:
        const = ctx.enter_context(tc.tile_pool(name="const", bufs=1))
        io = ctx.enter_context(tc.tile_pool(name="io", bufs=3))
        sa = ctx.enter_context(tc.tile_pool(name="sa", bufs=3))
        prodp = ctx.enter_context(tc.tile_pool(name="prodp", bufs=3))
        rhsp = ctx.enter_context(tc.tile_pool(name="rhsp", bufs=3))
        s3p = ctx.enter_context(tc.tile_pool(name="s3p", bufs=3))
        outp = ctx.enter_context(tc.tile_pool(name="outp", bufs=3))
        psT = ctx.enter_context(tc.tile_pool(name="psT", bufs=1, space="PSUM"))
        psY = ctx.enter_context(tc.tile_pool(name="psY", bufs=1, space="PSUM"))
        psD = ctx.enter_context(tc.tile_pool(name="psD", bufs=1, space="PSUM"))

        eps_c = const.tile([128, 1], F32, tag="eps_c")
        nc.gpsimd.memset(eps_c[:], EPS)

        # weights: f32 load once, cast to bf16 (one ACT pass)
        wsb = const.tile([128, 5, 128], F32)
        nc.sync.dma_start(wsb[:], WMM.rearrange("n p m -> p n m"))
        wbf = const.tile([128, 5, 128], BF16)
        nc.scalar.activation(wbf[:], wsb[:], AF.Copy)
        W0 = wbf[:, 0, :]      # [WaT;W2nT | WaT;W2T]  (cols 0:64 -> Y0, 64:128 -> Y2)
        W1 = wbf[:, 1, :]      # blkdiag(WbT, WbT)     (cols 0:64 -> Y0, 64:128 -> Y2)
        W2s = wbf[:, 2, 0:64]  # [WaT; -WbT]           (-> Y1)
        W3 = wbf[:, 3, :]      # blkdiag(WdT, WdT)
        IDb = wbf[:, 4, :]     # identity (bf16 transposes)

        for u in range(NSUP):
            xst = io.tile([128, NCHUNK * 192], BF16, tag="xs")
            jst = io.tile([128, NCHUNK * 192], BF16, tag="js")
            nc.sync.dma_start(jst[:], JS[u])
            nc.sync.dma_start(xst[:], XS[u])
            # host layout is [s, c, e]: every field slice is packed (innermost e)
            xv = xst[:].rearrange("p (s c e) -> p s c e", s=NCHUNK, c=3, e=E)
            jv = jst[:].rearrange("p (s c e) -> p s c e", s=NCHUNK, c=3, e=E)

            def v3(t):  # [128, 512] tile -> [128, 8, 64] view
                return t[:].rearrange("p (s e) -> p s e", s=NCHUNK, e=E)

            # ---- stage A: per-edge scalars gam, a1 ---------------------
            qn2 = sa.tile([128, 2, SUPER // 2], BF16, tag="qn2")
            q = qn2[:, 0, :]
            n2 = qn2[:, 1, :]
            nc.vector._custom_dve(OPS["SQSUM"], out=q,
                                  in0=jv[:, :, 0, :], in1=jv[:, :, 1, :]
                                  ).ins.perf_max = 2
            nc.vector._custom_dve(OPS["ADDSQ"], out=n2,
                                  in0=q, in1=jv[:, :, 2, :]
                                  ).ins.perf_max = 2
            sqs = sa.tile([128, 2, SUPER // 2], BF16, tag="sqs")
            nc.scalar.activation(sqs[:], qn2[:], AF.Sqrt)
            sq_ = sqs[:, 0, :]
            s_ = sqs[:, 1, :]
            gam = sa.tile([128, SUPER // 2], BF16, tag="gam")
            nc.gpsimd.tensor_tensor(v3(gam), jv[:, :, 2, :],
                                    s_.rearrange("p (s e) -> p s e", s=NCHUNK, e=E),
                                    ALU.divide)
            a1u = sa.tile([128, SUPER // 2], BF16, tag="a1u")
            nc.gpsimd.tensor_tensor(a1u[:], sq_, s_, ALU.divide)
            a1 = sa.tile([128, SUPER // 2], BF16, tag="a1")
            nc.vector._custom_dve(OPS["CPSGN"], out=a1[:], in0=a1u[:],
                                  in1=jv[:, :, 2, :], s0=EPS).ins.perf_max = 2

            # ---- stage B: products into paired prod slots --------------
            # slots: 0=Xx 1=p2 2=xyb 3=c8 4=Xz 5=p3 6=p4 7=p5
            prod = prodp.tile([128, NCHUNK, 8, E], BF16, tag="prod")
            nc.vector.tensor_copy(prod[:, :, 0:5:2, :], xv[:])
            m1 = sa.tile([128, SUPER // 2], BF16, tag="m1")
            m2 = sa.tile([128, SUPER // 2], BF16, tag="m2")
            m3 = sa.tile([128, SUPER // 2], BF16, tag="m3")
            m4 = sa.tile([128, SUPER // 2], BF16, tag="m4")
            nc.gpsimd.tensor_tensor(v3(m1), v3(gam), prod[:, :, 4, :], ALU.mult)
            nc.gpsimd.tensor_tensor(v3(m2), v3(a1), prod[:, :, 0, :], ALU.mult)
            nc.gpsimd.tensor_tensor(v3(m3), v3(a1), prod[:, :, 4, :], ALU.mult)
            nc.gpsimd.tensor_tensor(v3(m4), v3(gam), prod[:, :, 0, :], ALU.mult)
            wt = sa.tile([128, SUPER // 2], BF16, tag="wt")
            nc.vector.tensor_tensor(wt[:], m1[:], m2[:], ALU.subtract)
            nc.vector.tensor_tensor(prod[:, :, 3, :], v3(m3), v3(m4), ALU.add)
            nc.vector.tensor_tensor(prod[:, :, 1, :], v3(a1), v3(wt), ALU.mult)
            nc.vector.tensor_tensor(prod[:, :, 5, :], v3(gam), v3(wt), ALU.mult)
            nc.gpsimd.tensor_tensor(prod[:, :, 6, :], v3(gam), prod[:, :, 2, :],
                                    ALU.mult)
            nc.gpsimd.tensor_tensor(prod[:, :, 7, :], v3(a1), prod[:, :, 2, :],
                                    ALU.mult)

            # ---- per group: transpose, rh copies, Y matmuls ------------
            pY = psY.tile([128, 3, GROUP], F32, tag="pY")
            for g in range(2):
                tpA = psT.tile([128, 2, GROUP], BF16, tag="tpA")
                tpB = psT.tile([128, 2, GROUP], BF16, tag="tpB")
                for k in range(4):
                    s = 4 * g + k
                    sl = slice(128 * k, 128 * (k + 1))
                    nc.tensor.transpose(tpA[:, 0, sl], prod[:, s, 0:2, :], IDb)
                    nc.tensor.transpose(tpA[:, 1, sl], prod[:, s, 4:6, :], IDb)
                    nc.tensor.transpose(tpB[:, 0, sl], prod[:, s, 2:4, :], IDb)
                    nc.tensor.transpose(tpB[:, 1, sl], prod[:, s, 6:8, :], IDb)
                rhA = rhsp.tile([128, 2, GROUP], BF16, tag="rhA")
                rhB = rhsp.tile([128, 2, GROUP], BF16, tag="rhB")
                if g == 0:
                    nc.vector.tensor_copy(rhA[:], tpA[:])
                    nc.vector.tensor_copy(rhB[:], tpB[:])
                else:
                    nc.scalar.activation(rhA[:], tpA[:], AF.Copy)
                    nc.scalar.activation(rhB[:], tpB[:], AF.Copy)

                ro = slice(64 * g, 64 * (g + 1))
                rh1 = rhA[:, 0, :]   # [Xx; p2]
                rh2 = rhA[:, 1, :]   # [Xz; p3]
                rh3 = rhB[:, 0, :]   # [xyb; c8]
                rh4 = rhB[:, 1, :]   # [p4; p5]
                nc.tensor.matmul(pY[ro, 0, :], W0[:, 0:64], rh1,
                                 start=True, stop=False)
                nc.tensor.matmul(pY[ro, 0, :], W1[:, 0:64], rh4,
                                 start=False, stop=True)
                nc.tensor.matmul(pY[ro, 2, :], W0[:, 64:128], rh2,
                                 start=True, stop=False)
                nc.tensor.matmul(pY[ro, 2, :], W1[:, 64:128], rh4,
                                 start=False, stop=True)
                nc.tensor.matmul(pY[ro, 1, :], W2s, rh3, start=True, stop=True)

            # ---- Wd stage ----------------------------------------------
            xsb = s3p.tile([128, 3, GROUP], BF16, tag="xsb")
            nc.scalar.activation(xsb[:], pY[:], AF.Copy)
            pd = psD.tile([128, 3, GROUP], F32, tag="pd")
            for i in range(3):
                nc.tensor.matmul(pd[:, i, :], W3, xsb[:, i, :], start=True,
                                 stop=True)
            dsb = s3p.tile([128, 3, GROUP], BF16, tag="dsb")
            nc.scalar.activation(dsb[:], pd[:], AF.Copy)

            # ---- VN leaky relu -----------------------------------------
            P = s3p.tile([128, 3, GROUP], BF16, tag="P")
            nc.vector.tensor_tensor(P[:], xsb[:], dsb[:], ALU.mult)
            dot = s3p.tile([128, GROUP], BF16, tag="dot")
            nc.vector.tensor_tensor(dot[:], P[:, 0, :], P[:, 1, :], ALU.add)
            dotm = s3p.tile([128, GROUP], BF16, tag="dotm")
            nc.vector._custom_dve(OPS["MINADD"], out=dotm[:],
                                  in0=dot[:], in1=P[:, 2, :]).ins.perf_max = 2

            dn2 = s3p.tile([128, GROUP], BF16, tag="dn2")
            nc.vector._custom_dve(OPS["SQSUM"], out=dn2[:],
                                  in0=dsb[:, 0, :], in1=dsb[:, 1, :]
                                  ).ins.perf_max = 2
            dn2f = s3p.tile([128, GROUP], BF16, tag="dn2f")
            nc.vector._custom_dve(OPS["ADDSQSE"], out=dn2f[:],
                                  in0=dn2[:], in1=dsb[:, 2, :],
                                  s0=1.0 / (1.0 - NEG),
                                  s1=EPS / (1.0 - NEG)).ins.perf_max = 2
            s2 = s3p.tile([128, GROUP], BF16, tag="s2")
            nc.gpsimd.tensor_tensor(s2[:], dotm[:], dn2f[:], ALU.divide)

            mi3 = s3p.tile([128, 3, GROUP], BF16, tag="mi3")
            for i in range(3):
                nc.gpsimd.tensor_tensor(mi3[:, i, :], s2[:], dsb[:, i, :],
                                        ALU.mult)
            ot = outp.tile([128, 3, GROUP], BF16, tag="ot")
            nc.gpsimd.tensor_tensor(ot[:], xsb[:], mi3[:], ALU.subtract)

            c0 = u * SUPER
            nc.sync.dma_start(OUT[:, :, c0:c0 + GROUP], ot[0:64])
            nc.sync.dma_start(OUT[:, :, c0 + GROUP:c0 + SUPER], ot[64:128])

    nc.compile()
    return nc


_NC = None


def _get_nc():
    global _NC
    if _NC is None:
        _NC = _build_nc()
    return _NC


def _weight_stack(Wa, Wb, Wc, Wd):
    Z = np.zeros((64, 64), np.float32)
    WaT = Wa.T.astype(np.float32)
    WbT = Wb.T.astype(np.float32)
    W2nT = (Wa - Wc).T.astype(np.float32)
    W2T = (Wc - Wa).T.astype(np.float32)
    WdT = Wd.T.astype(np.float32)

    def vs(a, b):
        return np.vstack([a, b]).astype(np.float32)   # [128, 64]

    s0 = np.hstack([vs(WaT, W2nT), vs(WaT, W2T)])     # [128,128]
    s1 = np.block([[WbT, Z], [Z, WbT]]).astype(np.float32)
    s2 = np.hstack([vs(WaT, -WbT), np.zeros((128, 64), np.float32)])
    s3 = np.block([[WdT, Z], [Z, WdT]]).astype(np.float32)
    s4 = np.eye(128, dtype=np.float32)
    return np.ascontiguousarray(np.stack([s0, s1, s2, s3, s4]), np.float32)


def _prep_input(A):
    """[C, E, 3] f32 -> [NSUP, 128, NCHUNK*192] bf16, [s, c, e]-ordered per
    point so every field slice is packed, with each partition's 8 point-rows
    contiguous in HBM (3072B DMA descriptors)."""
    import ml_dtypes
    Ap = A.reshape(NSUP, NCHUNK, 128, E, 3).transpose(0, 2, 1, 4, 3)
    Ap = np.ascontiguousarray(Ap.reshape(NSUP, 128, NCHUNK * 192))
    return Ap.astype(ml_dtypes.bfloat16)


def run_full(X, J, Wa, Wb, Wc, Wd, trace=False, trace_kwargs=None):
    nc = _get_nc()
    wmm = _weight_stack(Wa, Wb, Wc, Wd)
    in_maps = []
    for b in range(B):
        in_maps.append({
            "XS": _prep_input(np.asarray(X[b], np.float32)),
            "JS": _prep_input(np.asarray(J[b], np.float32)),
            "WMM": wmm,
        })
    res = bass_utils.run_bass_kernel_spmd(
        nc, in_maps, core_ids=list(range(B)), trace=trace,
        **(trace_kwargs or {}))
    out = np.stack([np.asarray(res.results[b]["OUT"]).astype(np.float32)
                    for b in range(B)])
    return out, res


def kernel(X, J, Wa, Wb, Wc, Wd):
    out, _ = run_full(X, J, Wa, Wb, Wc, Wd)
    return out
